# revision 91
# baseline (speedup 1.0000x reference)
"""Trainium2 Bass kernel for nn_Attn_30623116820602.

Low-rank-projected causal multi-head attention:
  q/k/v = (x @ A) @ B  (rank 192), RoPE on q,k, causal softmax attention,
  output projection.  x: [128, 256, 768] fp32.

Sharding: pure data-parallel over batch (16 items per core, 8 cores).
Feature-major layout (d_model on partitions) throughout; host pre/post
transposes.  All matmul inputs are bf16 (PSUM accumulates fp32).

Structure (per pair of batch items = 512 token columns):
  - proj1 packs the 3 rank-192 outputs into 5 (not 6) 128-row tiles:
    [q0:128 | q128:192+k0:64 | k64:192 | v0:128 | v128:192+pad].
  - RoPE rotate-half comes from one extra matmul with a shared 128x128
    +-1 permutation matrix P (contraction 128) instead of duplicated
    rank-contraction weight matmuls.  (Partition-shift DMA variants
    measured slower: SBUF-to-SBUF DMA latency stalls the vector queue.)
  - Causal block structure is exploited: the fully-masked
    (keytile1 x querytile0) block is never computed -- not in scores,
    exp, denominators, nor the AV matmul.  E layout per (item, head) is
    [kt0q0 | kt0q1 | kt1q1] (one N=256 + one N=128 scores matmul); the
    two triangular blocks are masked with one [tril|ones|tril] multiply.
  - Softmax denominators: matmuls against an all-ones [128,128]
    stationary produce the denominator already replicated on all 128
    partitions (same column cost as a ones-vector), so the reciprocal
    runs as an efficient full-width [128,512] reciprocal_approx_fast
    and no partition broadcast is needed.  No DRAM round trip (the fp32
    baseline's 4-hop DRAM chain serialized the pipeline at ~16us/pair).
  - x loads are prefetched one pair ahead on the gpsimd DMA queue;
    output stores ride the sync queue; the output projection of pair
    N-1 is interleaved into pair N's attention to keep the PE busy.
"""

import math
import sys

sys.path.insert(0, "/opt/trn_rl_repo")

import numpy as np
import ml_dtypes


def _to_bf16(a):
    return a.astype(ml_dtypes.bfloat16)


B, T, D = 128, 256, 768
H, HD = 6, 128
RANK = 192
N_CORES = 8
B_LOC = B // N_CORES  # 16
N_PAIRS = B_LOC // 2  # 8 (2 batch items per pipeline iteration)
SCALE = 1.0 / math.sqrt(HD)

_CACHE = {}


def build_program(n_pairs=N_PAIRS):
    import concourse.tile as tile
    from concourse import bacc, mybir
    from contextlib import ExitStack

    f32 = mybir.dt.float32
    bf16 = mybir.dt.bfloat16
    TOK = n_pairs * 512

    nc = bacc.Bacc("TRN2", target_bir_lowering=False, debug=False,
                   num_devices=N_CORES)

    def din(name, shape):
        return nc.dram_tensor(name, shape, bf16, kind="ExternalInput").ap()

    xT = din("xT", [n_pairs, 128, 6, 512])
    # all weights ride in two packed blocks on the gpsimd hardware DMA
    # ring (the scalar-engine ring is the slow software_dynamic path);
    # block 1 = what pair 0 needs first.  x tile 0 rides the sync ring
    # in parallel.  Startup floor is ~8us of runtime/iram setup anyway.
    wpack1_l = din("wpack1_l", [128, 7552])   # A | qBp | kBp | P | cos | sin
    wpack2_l = din("wpack2_l", [128, 6656])   # vBp | mask | ones | ow
    outT = nc.dram_tensor("outT", [6, 128, TOK], f32, kind="ExternalOutput").ap()

    with tile.TileContext(nc) as tc:
        with ExitStack() as ctx:
            wp = ctx.enter_context(tc.tile_pool(name="w", bufs=1))
            xp = ctx.enter_context(tc.tile_pool(name="xt", bufs=2))
            xrp = ctx.enter_context(tc.tile_pool(name="xr", bufs=2))
            rawp = ctx.enter_context(tc.tile_pool(name="raw", bufs=2))
            qkp = ctx.enter_context(tc.tile_pool(name="qk", bufs=1))
            vp_ = ctx.enter_context(tc.tile_pool(name="vsb", bufs=2))
            tp = ctx.enter_context(tc.tile_pool(name="tmp", bufs=2))
            ep = ctx.enter_context(tc.tile_pool(name="eexp", bufs=8))
            dp = ctx.enter_context(tc.tile_pool(name="den", bufs=2))
            bp = ctx.enter_context(tc.tile_pool(name="bcast", bufs=2))
            orp = ctx.enter_context(tc.tile_pool(name="oraw", bufs=2))
            aop = ctx.enter_context(tc.tile_pool(name="ao", bufs=3))
            fp = ctx.enter_context(tc.tile_pool(name="fout", bufs=3))
            ps = ctx.enter_context(tc.tile_pool(name="ps", bufs=3, space="PSUM"))
            pm = ctx.enter_context(tc.tile_pool(name="pm", bufs=2, space="PSUM"))
            # rot (proj2 phase) and scores (attention phase) share one
            # 3-buffer pool; the phases don't overlap within a pair
            pq = ctx.enter_context(tc.tile_pool(name="pq", bufs=3, space="PSUM"))

            def psum():
                return ps.tile([128, 512], f32, tag="ps", name="psb")

            # ---- resident weights / constants: packed parallel DMAs ----
            # (x tile 0 rides the sync queue in parallel)
            xt0 = xp.tile([128, 6, 512], bf16, tag="xt", name="xt")
            nc.sync.dma_start(xt0[:], xT[0])
            wp1 = wp.tile([128, 7552], bf16, tag="wp1", name="wp1")
            nc.gpsimd.dma_start(wp1[:], wpack1_l)
            wp2 = wp.tile([128, 6656], bf16, tag="wp2", name="wp2")
            nc.gpsimd.dma_start(wp2[:], wpack2_l)
            A_s = wp1[:, 0:3840].rearrange("p (k m) -> p k m", k=6)
            qBp_s = wp1[:, 3840:5376].rearrange("p (k m) -> p k m", k=2)
            kBp_s = wp1[:, 5376:6912].rearrange("p (k m) -> p k m", k=2)
            P_s = wp1[:, 6912:7040]
            cos_s = wp1[:, 7040:7296]
            sin_s = wp1[:, 7296:7552]
            vBp_s = wp2[:, 0:1536].rearrange("p (k m) -> p k m", k=2)
            mask_s = wp2[:, 1536:1920]
            ones_s = wp2[:, 1920:2048]
            ow_s = wp2[:, 2048:6656].rearrange("p (k m) -> p k m", k=6)

            def emit_outproj(aosb_prev, pr_prev, mts, half=None, eng=None):
                w = 512 if half is None else 256
                c0 = 0 if half in (None, 0) else 256
                tokp = slice(pr_prev * 512 + c0, pr_prev * 512 + c0 + w)
                for mt in mts:
                    fps = psum()
                    for kt in range(6):
                        nc.tensor.matmul(
                            fps[:, 0:w],
                            ow_s[:, kt, mt * 128:(mt + 1) * 128],
                            aosb_prev[kt][:, c0:c0 + w],
                            start=(kt == 0), stop=(kt == 5))
                    fout = fp.tile([128, 512], f32, tag="fout", name="fout")
                    if eng is None:
                        nc.scalar.copy(fout[:, 0:w], fps[:, 0:w])
                    else:
                        eng.tensor_copy(fout[:, 0:w], fps[:, 0:w])
                    nc.sync.dma_start(outT[mt, :, tokp], fout[:, 0:w])

            # (xt0 was prefetched before the weight loads)
            xts = [None] * n_pairs
            xts[0] = xt0

            def load_xt(p):
                t = xp.tile([128, 6, 512], bf16, tag="xt", name="xt")
                nc.gpsimd.dma_start(t[:], xT[p])
                xts[p] = t

            prev = None
            for prx in range(n_pairs):
                if prx + 1 < n_pairs:
                    load_xt(prx + 1)
                xt = xts[prx]

                # ---- proj1: packed rank tiles [q|q+k|k|v|v] ----
                # accumulators come from the pq pool: its attention-phase
                # tiles (scores) are freed early by exp, so pair p+1's proj1
                # does not wait on pair p's trailing AV/norm chain
                xr = xrp.tile([128, 5, 512], bf16, tag="xr", name="xr")
                for rt in range(5):
                    mm = pq.tile([128, 512], f32, tag="pq", name="pq")
                    for kt in range(6):
                        nc.tensor.matmul(
                            mm[:],
                            A_s[:, kt, rt * 128:(rt + 1) * 128],
                            xt[:, kt, :],
                            start=(kt == 0), stop=(kt == 5))
                    nc.scalar.copy(xr[:, rt, :], mm[:])

                # ---- proj2 + RoPE for q and k (feature-major) ----
                # q contracts xr tiles {0,1}; k contracts {1,2} (B rows
                # zero-padded on host where tiles are shared).
                qsb = qkp.tile([128, 6, 512], bf16, tag="qsb", name="qsb")
                ksb = qkp.tile([128, 6, 512], bf16, tag="ksb", name="ksb")
                for h in range(6):
                    hc = slice(h * 128, (h + 1) * 128)
                    mains = {}
                    for pname, B_s, t0, sb in (
                            ("q", qBp_s, 0, qsb), ("k", kBp_s, 1, ksb)):
                        p_main = pm.tile([128, 512], f32, tag="pm", name="pm")
                        for kt in range(2):
                            nc.tensor.matmul(
                                p_main[:], B_s[:, kt, hc], xr[:, t0 + kt, :],
                                start=(kt == 0), stop=(kt == 1))
                        raw = rawp.tile([128, 512], bf16, tag=f"raw{pname}",
                                        name=f"raw{pname}")
                        nc.scalar.copy(raw[:], p_main[:])
                        mains[pname] = (raw, sb)
                    for pname in ("q", "k"):
                        raw, sb = mains[pname]
                        p_rot = pq.tile([128, 512], f32, tag="pq", name="pq")
                        nc.tensor.matmul(p_rot[:], P_s[:], raw[:],
                                         start=True, stop=True)
                        tmp = tp.tile([128, 512], bf16, tag="ropetmp",
                                      name="ropetmp")
                        nc.vector.tensor_tensor(
                            sb[:, h, :].rearrange("p (b q) -> p b q", b=2),
                            raw[:].rearrange("p (b q) -> p b q", b=2),
                            cos_s[:, None, :].to_broadcast((128, 2, 256)),
                            mybir.AluOpType.mult)
                        nc.vector.tensor_tensor(
                            tmp[:].rearrange("p (b q) -> p b q", b=2),
                            p_rot[:].rearrange("p (b q) -> p b q", b=2),
                            sin_s[:, None, :].to_broadcast((128, 2, 256)),
                            mybir.AluOpType.mult)
                        nc.vector.tensor_tensor(
                            sb[:, h, :], sb[:, h, :], tmp[:],
                            mybir.AluOpType.add)

                # ---- proj2 for v (token-major), contracts xr tiles {3,4} ----
                vsb = vp_.tile([128, 4, 768], bf16, tag="vsb", name="vsb")
                for mt in range(4):
                    for nch in range(2):
                        vps = psum()
                        for kt in range(2):
                            nc.tensor.matmul(
                                vps[:, 0:384],
                                xr[:, 3 + kt, mt * 128:(mt + 1) * 128],
                                vBp_s[:, kt, nch * 384:(nch + 1) * 384],
                                start=(kt == 0), stop=(kt == 1))
                        nc.scalar.copy(vsb[:, mt, nch * 384:(nch + 1) * 384],
                                       vps[:, 0:384])

                # ---- attention (per batch item) ----
                # E layout per (b, h): [kt0q0 | kt0q1 | kt1q1], each 128 wide.
                # per-head tiles so the output projection's kt-chain depends
                # on each head's norm individually, not the whole block
                aosb = [aop.tile([128, 512], bf16, tag=f"aosb{h}",
                                 name=f"aosb{h}") for h in range(6)]
                for b in range(2):
                    if prev is not None:
                        emit_outproj(prev[0], prev[1], range(3 * b, 3 * b + 3))
                    i_bD = bp.tile([128, 1536], f32, tag="ibD", name="ibD")
                    Es = []

                    def do_scores(h):
                        qh = qsb[:, h, b * 256:(b + 1) * 256]
                        kh = ksb[:, h, b * 256:(b + 1) * 256]
                        sp = pq.tile([128, 512], f32, tag="pq", name="pq")
                        nc.tensor.matmul(sp[:, 0:256], kh[:, 0:128],
                                         qh[:, 0:256], start=True, stop=True)
                        nc.tensor.matmul(sp[:, 256:384], kh[:, 128:256],
                                         qh[:, 128:256], start=True, stop=True)
                        if h % 2 == 0:
                            Es.append(ep.tile([128, 2, 384], bf16,
                                              tag="E", name="E"))
                        E = Es[h // 2][:, h % 2, :]
                        nc.scalar.activation(
                            E, sp[:, 0:384],
                            mybir.ActivationFunctionType.Exp, scale=SCALE)
                        # one mask multiply: [tril | ones | tril]
                        nc.vector.tensor_tensor(E, E, mask_s[:],
                                                mybir.AluOpType.mult)

                    def do_den(s):
                        # all-ones stationary replicates the key-sum across
                        # all 128 partitions (broadcast comes for free)
                        E2 = Es[s]
                        dps = psum()
                        for hh in range(2):
                            c = hh * 256
                            nc.tensor.matmul(dps[:, c:c + 256], ones_s[:],
                                             E2[:, hh, 0:256],
                                             start=True, stop=False,
                                             skip_group_check=True)
                            nc.tensor.matmul(dps[:, c + 128:c + 256], ones_s[:],
                                             E2[:, hh, 256:384],
                                             start=False, stop=True,
                                             skip_group_check=True)
                        sc = slice(s * 512, (s + 1) * 512)
                        nc.vector.reciprocal_approx_fast(i_bD[:, sc], dps[:, :])

                    def do_av(h):
                        # attention output, normalized straight out of PSUM
                        E2 = Es[h // 2]
                        o2 = psum()
                        v0 = vsb[:, b * 2 + 0, h * 128:(h + 1) * 128]
                        v1 = vsb[:, b * 2 + 1, h * 128:(h + 1) * 128]
                        nc.tensor.matmul(o2[:, 0:256], v0, E2[:, h % 2, 0:256],
                                         start=True, stop=False,
                                         skip_group_check=True)
                        nc.tensor.matmul(o2[:, 128:256], v1,
                                         E2[:, h % 2, 256:384],
                                         start=False, stop=True,
                                         skip_group_check=True)
                        nc.vector.tensor_tensor(
                            aosb[h][:, b * 256:(b + 1) * 256],
                            o2[:, 0:256],
                            i_bD[:, h * 256:(h + 1) * 256],
                            mybir.AluOpType.mult)

                    # one-group software lag keeps the tensor queue fed while
                    # exp/mask of the current group are still in flight
                    for s in range(3):
                        do_scores(2 * s)
                        do_scores(2 * s + 1)
                        if s >= 1:
                            do_den(s - 1)
                            do_av(2 * (s - 1))
                            do_av(2 * (s - 1) + 1)
                    do_den(2)
                    do_av(4)
                    do_av(5)
                    if prx == n_pairs - 1:
                        # b=0's fout copies go to vector so b=1's exps are
                        # not queued behind them on scalar; b=1's go to the
                        # by-then-idle scalar engine
                        emit_outproj(aosb, prx, range(6), half=b,
                                     eng=nc.vector if b == 0 else None)
                prev = (aosb, prx)

    nc.compile()
    return nc


def _rope_tables():
    inv = 1.0 / (10000.0 ** (np.arange(0, HD, 2, dtype=np.float32) / HD))
    t = np.arange(T, dtype=np.float32)
    freqs = np.outer(t, inv)                      # [T, 64]
    emb = np.concatenate([freqs, freqs], axis=-1)  # [T, 128]
    return np.cos(emb).astype(np.float32), np.sin(emb).astype(np.float32)


def _prep_shared(qA, qB, kA, kB, vA, vB, o_w):
    """Host-side weight/constant layouts (shared by all cores)."""
    def a_r(A):  # [768,192] -> [6,128,192]
        return A.reshape(6, 128, RANK)

    qA_r, kA_r, vA_r = a_r(qA), a_r(kA), a_r(vA)
    Ap = np.zeros((6, 128, 640), np.float32)
    Ap[:, :, 0:128] = qA_r[:, :, 0:128]
    Ap[:, :, 128:192] = qA_r[:, :, 128:192]
    Ap[:, :, 192:256] = kA_r[:, :, 0:64]
    Ap[:, :, 256:384] = kA_r[:, :, 64:192]
    Ap[:, :, 384:512] = vA_r[:, :, 0:128]
    Ap[:, :, 512:576] = vA_r[:, :, 128:192]

    qBp = np.zeros((2, 128, D), np.float32)
    qBp[0] = qB[0:128]
    qBp[1, 0:64] = qB[128:192]

    kBp = np.zeros((2, 128, D), np.float32)
    kBp[0, 64:128] = kB[0:64]
    kBp[1] = kB[64:192]

    vBp = np.zeros((2, 128, D), np.float32)
    vBp[0] = vB[0:128]
    vBp[1, 0:64] = vB[128:192]

    # rotate-half permutation (as matmul lhsT): out[m] = sum_k P[k,m] q[k]
    P = np.zeros((128, 128), np.float32)
    for m in range(64):
        P[m + 64, m] = -1.0
        P[m, m + 64] = 1.0

    cos, sin = _rope_tables()
    cosT = np.ascontiguousarray(cos.T)  # [128, 256]
    sinT = np.ascontiguousarray(sin.T)

    p = np.arange(128)[:, None]
    j = np.arange(128)[None, :]
    tril = (p <= j).astype(np.float32)  # keys (partitions) <= queries (cols)
    mask = np.concatenate(
        [tril, np.ones((128, 128), np.float32), tril], axis=1)  # [128, 384]

    wpack1 = np.concatenate([
        Ap.transpose(1, 0, 2).reshape(128, 3840),
        qBp.transpose(1, 0, 2).reshape(128, 1536),
        kBp.transpose(1, 0, 2).reshape(128, 1536),
        P, cosT, sinT,
    ], axis=1)  # [128, 7552]
    wpack2 = np.concatenate([
        vBp.transpose(1, 0, 2).reshape(128, 1536),
        mask,
        np.ones((128, 128), np.float32),
        o_w.reshape(6, 128, D).transpose(1, 0, 2).reshape(128, 4608),
    ], axis=1)  # [128, 6656]

    return {
        "wpack1_l": _to_bf16(np.ascontiguousarray(wpack1)),
        "wpack2_l": _to_bf16(np.ascontiguousarray(wpack2)),
    }


def x_to_xT(xc):
    """[2*n_pairs, T, D] -> [n_pairs, 128, 6, 512] feature-major tiles."""
    nb = xc.shape[0]
    return np.ascontiguousarray(
        _to_bf16(xc).reshape(nb // 2, 2, T, 6, 128).transpose(0, 4, 3, 1, 2)
        .reshape(nb // 2, 128, 6, 512))


def outT_to_out(oT, nb):
    return np.ascontiguousarray(
        oT.reshape(6, 128, nb, T).transpose(2, 3, 0, 1).reshape(nb, T, D))


def kernel(x, qA, qB, kA, kB, vA, vB, o_w):
    from concourse import bass_utils

    if "nc" not in _CACHE:
        _CACHE["nc"] = build_program(N_PAIRS)
    nc = _CACHE["nc"]

    shared = _prep_shared(
        np.asarray(qA, np.float32), np.asarray(qB, np.float32),
        np.asarray(kA, np.float32), np.asarray(kB, np.float32),
        np.asarray(vA, np.float32), np.asarray(vB, np.float32),
        np.asarray(o_w, np.float32))
    x = np.asarray(x, np.float32)

    in_maps = []
    for c in range(N_CORES):
        m = dict(shared)
        m["xT"] = x_to_xT(x[c * B_LOC:(c + 1) * B_LOC])
        in_maps.append(m)

    res = bass_utils.run_bass_kernel_spmd(
        nc, in_maps, core_ids=list(range(N_CORES)))
    out = np.empty((B, T, D), np.float32)
    for c in range(N_CORES):
        out[c * B_LOC:(c + 1) * B_LOC] = outT_to_out(
            res.results[c]["outT"], B_LOC)
    return out
